# revision 10
# baseline (speedup 1.0000x reference)
"""LinkPredictor similarity kernel for 8 Trainium2 NeuronCores.

reference:
    sims = E @ E.T               # [16384, 16384], E = [16384, 512] fp32
    m, M = sims.min(), sims.max()
    sims = (sims - m) / (M - m + 1e-7)
    out  = sims[row_idx, col_idx]     # block-diag strict-upper-tri gather

Only the 128 diagonal [128,128] graph blocks are ever gathered, but the
global min needs every entry of sims. Two mathematical shortcuts:
  * sims is symmetric -> min over the block upper triangle suffices.
  * By Cauchy-Schwarz, s_ij <= |e_i||e_j| <= max_k |e_k|^2 = max diag,
    so the global max is exactly the max diagonal entry -- free from the
    bf16 diagonal blocks; the expensive sweep only tracks the min.

Distribution: 16 half-slabs of 1024 rows; core c owns half-slabs
{c, 15-c} and the 17 upper-triangle [1024,1024] blocks whose row
half-slab is one of those (every core gets exactly 17 blocks).

The sweep runs fp8e4 (e4m3) matmuls in DoubleRow perf mode (K=256 per
instruction): per [1024,1024] block, 8 two-bank PSUM pairs [128,1024]
are each filled by 4 matmuls. The running elementwise min is split to
keep both elementwise engines below the TensorE pace: per 8 pairs, 6 go
ScalarE-copy(fp16) -> VectorE running tensor_tensor min (two alternating
accumulators to break the RAW chain), 2 go VectorE tensor_reduce-min
directly on fp32 PSUM into independent slot columns. The 16 bf16
diagonal graph blocks run at the end (overlapping the VectorE drain)
and DMA straight out of PSUM. Host-simulated numerics: total pipeline
rel err ~1.6e-3 vs the 2e-2 gate.

Host combines mins, takes max from the block diagonals, normalizes and
gathers with the real row/col indices.
"""

import numpy as np
import ml_dtypes

N_GRAPHS = 128
G = 128
D = 512
N = N_GRAPHS * G          # 16384
EPS = 1e-7
NCORES = 8
HS = 1024                 # half-slab rows
NHS = N // HS             # 16 half-slabs
NBLK = 17                 # triangle blocks per core
KC = D // 128             # 4 contraction chunks of 128
MT = HS // 128            # 8 m-tiles per block
GPC = 16                  # graphs per core

_CACHED = {}
LAST_RESULTS = None       # BassKernelResults of the most recent run

# per-8-pair routing: 'a' = scalar->fp16->vector TT min, 'c' = vector
# tensor_reduce min direct on fp32 PSUM into a slot column
ROUTE = "aacaacaa"
NSLOTS = 64               # 2 'c' pairs x 17 blocks = 34 used


def _build_program():
    import concourse.bacc as bacc
    import concourse.mybir as mybir
    from concourse.tile import TileContext

    f32 = mybir.dt.float32
    f16 = mybir.dt.float16
    bf16 = mybir.dt.bfloat16
    f8 = mybir.dt.float8e4
    DR = mybir.MatmulPerfMode.DoubleRow
    MIN = mybir.AluOpType.min

    nc = bacc.Bacc(target_bir_lowering=False)
    # per-partition-contiguous packing: [block, partition, d1, col]
    lhs = nc.declare_dram_parameter("lhs", [NBLK, 128, KC, HS], f8, isOutput=False)
    rhs = nc.declare_dram_parameter("rhs", [NBLK, 128, KC, HS], f8, isOutput=False)
    dg_in = nc.declare_dram_parameter("dg", [128, KC, GPC * G], bf16, isOutput=False)
    diag_out = nc.declare_dram_parameter("diag_out", [GPC, G, G], f32, isOutput=True)
    mins = nc.declare_dram_parameter("mins", [128, 3], f32, isOutput=True)

    with TileContext(nc) as tc:
        with (
            tc.tile_pool(name="stream", bufs=6) as stream,
            tc.tile_pool(name="small", bufs=4) as small,
            tc.tile_pool(name="cpp", bufs=3) as cpp,
            tc.tile_pool(name="acc", bufs=1) as accp,
            tc.tile_pool(name="ps", bufs=3, space="PSUM") as ps,
            tc.tile_pool(name="psd", bufs=2, space="PSUM") as psd,
        ):
            run_min = [
                accp.tile([128, 1024], f16, tag=f"run_min{i}", name=f"run_min{i}")
                for i in range(2)
            ]
            slots = accp.tile([128, NSLOTS], f32, tag="slots")
            nc.vector.memset(run_min[0][:], 60000.0)
            nc.vector.memset(run_min[1][:], 60000.0)
            nc.vector.memset(slots[:], 3.0e38)

            dgt = accp.tile([128, KC, GPC * G], bf16, tag="dgt")

            na = 0
            nslot = 0
            for b in range(NBLK):
                lt = stream.tile([128, KC, HS], f8, tag="lt")
                rt = stream.tile([128, KC, HS], f8, tag="rt")
                # two HW-DGE rings: lt on the SP ring, rt on the ACT ring
                if b == 0:
                    # fast start: land the m=0 operands first
                    nc.scalar.dma_start(out=rt[:, :, 0:512], in_=rhs[b][:, :, 0:512])
                    nc.sync.dma_start(out=lt[:, :, 0:256], in_=lhs[b][:, :, 0:256])
                    nc.scalar.dma_start(
                        out=rt[:, :, 512:HS], in_=rhs[b][:, :, 512:HS]
                    )
                    nc.sync.dma_start(
                        out=lt[:, :, 256:HS], in_=lhs[b][:, :, 256:HS]
                    )
                else:
                    nc.sync.dma_start(out=lt[:], in_=lhs[b])
                    nc.scalar.dma_start(out=rt[:], in_=rhs[b])
                if b == 2:
                    # diag operands: consumed only at the end
                    nc.sync.dma_start(out=dgt[:], in_=dg_in[:])

                for m in range(MT):
                    acc = ps.tile([128, 1024], f32, tag="acc")
                    for n in range(2):
                        for k2 in range(2):
                            nc.tensor.matmul(
                                acc[:, n * 512 : (n + 1) * 512],
                                lt[:, 2 * k2 : 2 * k2 + 2, m * 128 : (m + 1) * 128],
                                rt[:, 2 * k2 : 2 * k2 + 2, n * 512 : (n + 1) * 512],
                                start=(k2 == 0), stop=(k2 == 1),
                                perf_mode=DR,
                            )
                    if ROUTE[m] == "a":
                        cp = cpp.tile([128, 1024], f16, tag="cpv")
                        nc.scalar.copy(cp[:], acc[:])
                        rm = run_min[na % 2]
                        na += 1
                        nc.vector.tensor_tensor(rm[:], rm[:], cp[:], MIN)
                    else:
                        nc.vector.tensor_reduce(
                            slots[:, nslot : nslot + 1], acc[:],
                            mybir.AxisListType.X, MIN,
                        )
                        nslot += 1

            # --- bf16 diagonal graph blocks at the end (overlap DVE drain) ---
            for g in range(GPC):
                dacc = psd.tile([128, G], f32, tag="dacc")
                for k in range(KC):
                    nc.tensor.matmul(
                        dacc[:],
                        dgt[:, k, g * G : (g + 1) * G],
                        dgt[:, k, g * G : (g + 1) * G],
                        start=(k == 0), stop=(k == KC - 1),
                    )
                dcp = small.tile([128, G], f32, tag="dcp")
                if g % 2 == 0:
                    nc.scalar.copy(dcp[:], dacc[:])
                else:
                    nc.vector.tensor_copy(dcp[:], dacc[:])
                nc.sync.dma_start(out=diag_out[g], in_=dcp[:])

            mm = small.tile([128, 3], f32, tag="mm")
            nc.vector.tensor_reduce(
                mm[:, 0:1], run_min[0][:], mybir.AxisListType.X, MIN
            )
            nc.vector.tensor_reduce(
                mm[:, 1:2], run_min[1][:], mybir.AxisListType.X, MIN
            )
            nc.vector.tensor_reduce(
                mm[:, 2:3], slots[:], mybir.AxisListType.X, MIN
            )
            nc.sync.dma_start(out=mins[:], in_=mm[:])

    nc.finalize()
    return nc


def _core_items(c: int):
    rows = [c, NHS - 1 - c]
    items = [(i, j) for i in rows for j in range(i, NHS)]
    assert len(items) == NBLK
    return items


def kernel(embeddings, row_idx, col_idx):
    global LAST_RESULTS
    from concourse.bass_utils import run_bass_kernel_spmd

    emb = np.asarray(embeddings, dtype=np.float32)
    row_idx = np.asarray(row_idx)
    col_idx = np.asarray(col_idx)

    if "nc" not in _CACHED:
        _CACHED["nc"] = _build_program()
    nc = _CACHED["nc"]

    eT = np.ascontiguousarray(emb.T)                       # [512, 16384] fp32
    e8 = eT.astype(ml_dtypes.float8_e4m3)                  # e4m3, RTNE
    # [slab, partition, d1, col]: per-partition-contiguous 4KB lines
    p8 = np.ascontiguousarray(
        e8.reshape(KC, 128, NHS, HS).transpose(2, 1, 0, 3)
    )
    e16 = eT.astype(ml_dtypes.bfloat16).reshape(KC, 128, NHS, HS)

    in_maps = []
    for c in range(NCORES):
        items = _core_items(c)
        lhs = p8[[i for i, _ in items]]
        rhs = p8[[j for _, j in items]]
        dg = np.ascontiguousarray(
            e16[:, :, [c, NHS - 1 - c], :].transpose(1, 0, 2, 3).reshape(
                128, KC, GPC * G
            )
        )
        in_maps.append({"lhs": lhs, "rhs": rhs, "dg": dg})

    res = run_bass_kernel_spmd(nc, in_maps, list(range(NCORES)))
    LAST_RESULTS = res

    m = min(r["mins"].min() for r in res.results)

    blocks = np.empty((N_GRAPHS, G, G), np.float32)
    gph = HS // G  # graphs per half-slab = 8
    for c in range(NCORES):
        rows = [c, NHS - 1 - c]
        gids = [i * gph + k for i in rows for k in range(gph)]
        for idx, g in enumerate(gids):
            blocks[g] = res.results[c]["diag_out"][idx]

    M = np.einsum("gii->gi", blocks).max()                 # global max (Cauchy-Schwarz)

    norm = (blocks - m) / (M - m + EPS)
    r = row_idx.astype(np.int64)
    cc = col_idx.astype(np.int64)
    out = norm[r >> 7, r & 127, cc & 127].astype(np.float32)
    return out


# revision 11
# speedup vs baseline: 1.2635x; 1.2635x over previous
"""LinkPredictor similarity kernel for 8 Trainium2 NeuronCores.

reference:
    sims = E @ E.T               # [16384, 16384], E = [16384, 512] fp32
    m, M = sims.min(), sims.max()
    sims = (sims - m) / (M - m + 1e-7)
    out  = sims[row_idx, col_idx]     # block-diag strict-upper-tri gather

Only the 128 diagonal [128,128] graph blocks are ever gathered, but the
global min needs every entry of sims. Two mathematical shortcuts:
  * sims is symmetric -> min over the block upper triangle suffices.
  * By Cauchy-Schwarz, s_ij <= |e_i||e_j| <= max_k |e_k|^2 = max diag,
    so the global max is exactly the max diagonal entry -- free from the
    bf16 diagonal blocks; the expensive sweep only tracks the min.

Distribution: 16 half-slabs of 1024 rows; core c owns half-slabs
{c, 15-c} and the 17 upper-triangle [1024,1024] blocks whose row
half-slab is one of those (every core gets exactly 17 blocks).

The sweep runs fp8e4 (e4m3) matmuls in DoubleRow perf mode (K=256 per
instruction): per [1024,1024] block, 8 two-bank PSUM pairs [128,1024]
are each filled by 4 matmuls. The running elementwise min is split to
keep both elementwise engines below the TensorE pace: per 8 pairs, 6 go
ScalarE-copy(fp16) -> VectorE running tensor_tensor min (two alternating
accumulators to break the RAW chain), 2 go VectorE tensor_reduce-min
directly on fp32 PSUM into independent slot columns. The 16 bf16
diagonal graph blocks run at the end (overlapping the VectorE drain)
and DMA straight out of PSUM. Host-simulated numerics: total pipeline
rel err ~1.6e-3 vs the 2e-2 gate.

Host combines mins, takes max from the block diagonals, normalizes and
gathers with the real row/col indices.
"""

import numpy as np
import ml_dtypes

N_GRAPHS = 128
G = 128
D = 512
N = N_GRAPHS * G          # 16384
EPS = 1e-7
NCORES = 8
HS = 1024                 # half-slab rows
NHS = N // HS             # 16 half-slabs
NBLK = 17                 # triangle blocks per core
KC = D // 128             # 4 contraction chunks of 128
MT = HS // 128            # 8 m-tiles per block
GPC = 16                  # graphs per core

_CACHED = {}
LAST_RESULTS = None       # BassKernelResults of the most recent run

# per-8-pair routing: 'a' = scalar->fp16->vector TT min, 'c' = vector
# tensor_reduce min direct on fp32 PSUM into a slot column
ROUTE = "aacaacaa"
NSLOTS = 64               # 2 'c' pairs x 17 blocks = 34 used


def _build_program():
    import concourse.bacc as bacc
    import concourse.mybir as mybir
    from concourse.tile import TileContext

    f32 = mybir.dt.float32
    f16 = mybir.dt.float16
    bf16 = mybir.dt.bfloat16
    f8 = mybir.dt.float8e4
    DR = mybir.MatmulPerfMode.DoubleRow
    MIN = mybir.AluOpType.min

    nc = bacc.Bacc(target_bir_lowering=False)
    # per-partition-contiguous packing: [block, partition, d1, col]
    lhs = nc.declare_dram_parameter("lhs", [NBLK, 128, KC, HS], f8, isOutput=False)
    rhs = nc.declare_dram_parameter("rhs", [NBLK, 128, KC, HS], f8, isOutput=False)
    dg_in = nc.declare_dram_parameter("dg", [128, KC, GPC * G], bf16, isOutput=False)
    diag_out = nc.declare_dram_parameter("diag_out", [GPC, G, G], f32, isOutput=True)
    mins = nc.declare_dram_parameter("mins", [128, 3], f32, isOutput=True)

    with TileContext(nc) as tc:
        with (
            tc.tile_pool(name="stream", bufs=1) as stream,
            tc.tile_pool(name="small", bufs=4) as small,
            tc.tile_pool(name="cpp", bufs=3) as cpp,
            tc.tile_pool(name="acc", bufs=1) as accp,
            tc.tile_pool(name="ps", bufs=3, space="PSUM") as ps,
            tc.tile_pool(name="psd", bufs=2, space="PSUM") as psd,
        ):
            run_min = [
                accp.tile([128, 1024], f16, tag=f"run_min{i}", name=f"run_min{i}")
                for i in range(2)
            ]
            slots = accp.tile([128, NSLOTS], f32, tag="slots")
            nc.vector.memset(run_min[0][:], 60000.0)
            nc.vector.memset(run_min[1][:], 60000.0)
            nc.vector.memset(slots[:], 3.0e38)

            dgt = accp.tile([128, KC, GPC * G], bf16, tag="dgt")

            # fully-resident streams: one buffer per block, every input DMA
            # issued up front on the SP ring -- no ring-reuse waits, nothing
            # ever blocks behind compute in an engine queue.
            lts, rts = [], []
            for b in range(NBLK):
                lt = stream.tile([128, KC, HS], f8, tag=f"lt{b}", name=f"lt{b}")
                rt = stream.tile([128, KC, HS], f8, tag=f"rt{b}", name=f"rt{b}")
                lts.append(lt)
                rts.append(rt)
                if b == 0:
                    # fast start: land the m=0 operands first
                    nc.sync.dma_start(out=rt[:, :, 0:512], in_=rhs[b][:, :, 0:512])
                    nc.sync.dma_start(out=lt[:, :, 0:256], in_=lhs[b][:, :, 0:256])
                    nc.sync.dma_start(
                        out=rt[:, :, 512:HS], in_=rhs[b][:, :, 512:HS]
                    )
                    nc.sync.dma_start(
                        out=lt[:, :, 256:HS], in_=lhs[b][:, :, 256:HS]
                    )
                else:
                    nc.sync.dma_start(out=lt[:], in_=lhs[b])
                    nc.sync.dma_start(out=rt[:], in_=rhs[b])
                if b == 2:
                    # diag operands: consumed only at the end
                    nc.sync.dma_start(out=dgt[:], in_=dg_in[:])

            na = 0
            nslot = 0
            for b in range(NBLK):
                lt = lts[b]
                rt = rts[b]
                for m in range(MT):
                    acc = ps.tile([128, 1024], f32, tag="acc")
                    for n in range(2):
                        for k2 in range(2):
                            nc.tensor.matmul(
                                acc[:, n * 512 : (n + 1) * 512],
                                lt[:, 2 * k2 : 2 * k2 + 2, m * 128 : (m + 1) * 128],
                                rt[:, 2 * k2 : 2 * k2 + 2, n * 512 : (n + 1) * 512],
                                start=(k2 == 0), stop=(k2 == 1),
                                perf_mode=DR,
                            )
                    if ROUTE[m] == "a":
                        cp = cpp.tile([128, 1024], f16, tag="cpv")
                        nc.scalar.copy(cp[:], acc[:])
                        rm = run_min[na % 2]
                        na += 1
                        nc.vector.tensor_tensor(rm[:], rm[:], cp[:], MIN)
                    else:
                        nc.vector.tensor_reduce(
                            slots[:, nslot : nslot + 1], acc[:],
                            mybir.AxisListType.X, MIN,
                        )
                        nslot += 1

            # --- bf16 diagonal graph blocks at the end (overlap DVE drain) ---
            for g in range(GPC):
                dacc = psd.tile([128, G], f32, tag="dacc")
                for k in range(KC):
                    nc.tensor.matmul(
                        dacc[:],
                        dgt[:, k, g * G : (g + 1) * G],
                        dgt[:, k, g * G : (g + 1) * G],
                        start=(k == 0), stop=(k == KC - 1),
                    )
                dcp = small.tile([128, G], f32, tag="dcp")
                nc.scalar.copy(dcp[:], dacc[:])
                nc.sync.dma_start(out=diag_out[g], in_=dcp[:])

            mm = small.tile([128, 3], f32, tag="mm")
            nc.vector.tensor_reduce(
                mm[:, 0:1], run_min[0][:], mybir.AxisListType.X, MIN
            )
            nc.vector.tensor_reduce(
                mm[:, 1:2], run_min[1][:], mybir.AxisListType.X, MIN
            )
            nc.vector.tensor_reduce(
                mm[:, 2:3], slots[:], mybir.AxisListType.X, MIN
            )
            nc.sync.dma_start(out=mins[:], in_=mm[:])

    nc.finalize()
    return nc


def _core_items(c: int):
    rows = [c, NHS - 1 - c]
    items = [(i, j) for i in rows for j in range(i, NHS)]
    assert len(items) == NBLK
    return items


def kernel(embeddings, row_idx, col_idx):
    global LAST_RESULTS
    from concourse.bass_utils import run_bass_kernel_spmd

    emb = np.asarray(embeddings, dtype=np.float32)
    row_idx = np.asarray(row_idx)
    col_idx = np.asarray(col_idx)

    if "nc" not in _CACHED:
        _CACHED["nc"] = _build_program()
    nc = _CACHED["nc"]

    eT = np.ascontiguousarray(emb.T)                       # [512, 16384] fp32
    e8 = eT.astype(ml_dtypes.float8_e4m3)                  # e4m3, RTNE
    # [slab, partition, d1, col]: per-partition-contiguous 4KB lines
    p8 = np.ascontiguousarray(
        e8.reshape(KC, 128, NHS, HS).transpose(2, 1, 0, 3)
    )
    e16 = eT.astype(ml_dtypes.bfloat16).reshape(KC, 128, NHS, HS)

    in_maps = []
    for c in range(NCORES):
        items = _core_items(c)
        lhs = p8[[i for i, _ in items]]
        rhs = p8[[j for _, j in items]]
        dg = np.ascontiguousarray(
            e16[:, :, [c, NHS - 1 - c], :].transpose(1, 0, 2, 3).reshape(
                128, KC, GPC * G
            )
        )
        in_maps.append({"lhs": lhs, "rhs": rhs, "dg": dg})

    res = run_bass_kernel_spmd(nc, in_maps, list(range(NCORES)))
    LAST_RESULTS = res

    m = min(r["mins"].min() for r in res.results)

    blocks = np.empty((N_GRAPHS, G, G), np.float32)
    gph = HS // G  # graphs per half-slab = 8
    for c in range(NCORES):
        rows = [c, NHS - 1 - c]
        gids = [i * gph + k for i in rows for k in range(gph)]
        for idx, g in enumerate(gids):
            blocks[g] = res.results[c]["diag_out"][idx]

    M = np.einsum("gii->gi", blocks).max()                 # global max (Cauchy-Schwarz)

    norm = (blocks - m) / (M - m + EPS)
    r = row_idx.astype(np.int64)
    cc = col_idx.astype(np.int64)
    out = norm[r >> 7, r & 127, cc & 127].astype(np.float32)
    return out


# revision 12
# speedup vs baseline: 1.2913x; 1.0220x over previous
"""LinkPredictor similarity kernel for 8 Trainium2 NeuronCores.

reference:
    sims = E @ E.T               # [16384, 16384], E = [16384, 512] fp32
    m, M = sims.min(), sims.max()
    sims = (sims - m) / (M - m + 1e-7)
    out  = sims[row_idx, col_idx]     # block-diag strict-upper-tri gather

Only the 128 diagonal [128,128] graph blocks are ever gathered, but the
global min needs every entry of sims. Two mathematical shortcuts:
  * sims is symmetric -> min over the block upper triangle suffices.
  * By Cauchy-Schwarz, s_ij <= |e_i||e_j| <= max_k |e_k|^2 = max diag,
    so the global max is exactly the max diagonal entry -- free from the
    bf16 diagonal blocks; the expensive sweep only tracks the min.

Distribution: 16 half-slabs of 1024 rows; core c owns half-slabs
{c, 15-c} and the 17 upper-triangle [1024,1024] blocks whose row
half-slab is one of those (every core gets exactly 17 blocks).

The sweep runs fp8e4 (e4m3) matmuls in DoubleRow perf mode (K=256 per
instruction): per [1024,1024] block, 8 two-bank PSUM pairs [128,1024]
are each filled by 4 matmuls. The running elementwise min is split to
keep both elementwise engines below the TensorE pace: per 8 pairs, 6 go
ScalarE-copy(fp16) -> VectorE running tensor_tensor min (two alternating
accumulators to break the RAW chain), 2 go VectorE tensor_reduce-min
directly on fp32 PSUM into independent slot columns. The 16 bf16
diagonal graph blocks run at the end (overlapping the VectorE drain)
and DMA straight out of PSUM. Host-simulated numerics: total pipeline
rel err ~1.6e-3 vs the 2e-2 gate.

Host combines mins, takes max from the block diagonals, normalizes and
gathers with the real row/col indices.
"""

import numpy as np
import ml_dtypes

N_GRAPHS = 128
G = 128
D = 512
N = N_GRAPHS * G          # 16384
EPS = 1e-7
NCORES = 8
HS = 1024                 # half-slab rows
NHS = N // HS             # 16 half-slabs
NBLK = 17                 # triangle blocks per core
KC = D // 128             # 4 contraction chunks of 128
MT = HS // 128            # 8 m-tiles per block
GPC = 16                  # graphs per core

_CACHED = {}
LAST_RESULTS = None       # BassKernelResults of the most recent run

# per-8-pair routing: 'a' = scalar->fp16->vector TT min, 'c' = vector
# tensor_reduce min direct on fp32 PSUM into a slot column
ROUTE = "aacaacaa"
NSLOTS = 64               # 2 'c' pairs x 17 blocks = 34 used


def _build_program():
    import concourse.bacc as bacc
    import concourse.mybir as mybir
    from concourse.tile import TileContext

    f32 = mybir.dt.float32
    f16 = mybir.dt.float16
    bf16 = mybir.dt.bfloat16
    f8 = mybir.dt.float8e4
    DR = mybir.MatmulPerfMode.DoubleRow
    MIN = mybir.AluOpType.min

    nc = bacc.Bacc(target_bir_lowering=False)
    # per-partition-contiguous packing: [block, partition, d1, col]
    lhs = nc.declare_dram_parameter("lhs", [NBLK, 128, KC, HS], f8, isOutput=False)
    rhs = nc.declare_dram_parameter("rhs", [NBLK, 128, KC, HS], f8, isOutput=False)
    dg_in = nc.declare_dram_parameter("dg", [128, KC, GPC * G], bf16, isOutput=False)
    diag_out = nc.declare_dram_parameter("diag_out", [GPC, G, G], f32, isOutput=True)
    mins = nc.declare_dram_parameter("mins", [128, 3], f32, isOutput=True)

    with TileContext(nc) as tc:
        with (
            tc.tile_pool(name="stream", bufs=1) as stream,
            tc.tile_pool(name="small", bufs=4) as small,
            tc.tile_pool(name="cpp", bufs=3) as cpp,
            tc.tile_pool(name="acc", bufs=1) as accp,
            tc.tile_pool(name="ps", bufs=3, space="PSUM") as ps,
            tc.tile_pool(name="psd", bufs=2, space="PSUM") as psd,
        ):
            run_min = [
                accp.tile([128, 1024], f16, tag=f"run_min{i}", name=f"run_min{i}")
                for i in range(2)
            ]
            slots = accp.tile([128, NSLOTS], f32, tag="slots")
            nc.vector.memset(run_min[0][:], 60000.0)
            nc.vector.memset(run_min[1][:], 60000.0)
            nc.vector.memset(slots[:], 3.0e38)

            dgt = accp.tile([128, KC, GPC * G], bf16, tag="dgt")

            # fully-resident streams: one buffer per block, every input DMA
            # issued up front on the SP ring -- no ring-reuse waits, nothing
            # ever blocks behind compute in an engine queue.
            lts, rts = [], []
            for b in range(NBLK):
                lt = stream.tile([128, KC, HS], f8, tag=f"lt{b}", name=f"lt{b}")
                rt = stream.tile([128, KC, HS], f8, tag=f"rt{b}", name=f"rt{b}")
                lts.append(lt)
                rts.append(rt)
                if b == 0:
                    # fast start: land the m=0 operands first, rt via the
                    # ACT ring (its queue is empty this early) in parallel
                    nc.scalar.dma_start(out=rt[:, :, 0:512], in_=rhs[b][:, :, 0:512])
                    nc.sync.dma_start(out=lt[:, :, 0:256], in_=lhs[b][:, :, 0:256])
                    nc.scalar.dma_start(
                        out=rt[:, :, 512:HS], in_=rhs[b][:, :, 512:HS]
                    )
                    nc.sync.dma_start(
                        out=lt[:, :, 256:HS], in_=lhs[b][:, :, 256:HS]
                    )
                else:
                    nc.sync.dma_start(out=lt[:], in_=lhs[b])
                    nc.sync.dma_start(out=rt[:], in_=rhs[b])
                if b == 2:
                    # diag operands: consumed only at the end
                    nc.sync.dma_start(out=dgt[:], in_=dg_in[:])

            na = 0
            nslot = 0
            for b in range(NBLK):
                lt = lts[b]
                rt = rts[b]
                for m in range(MT):
                    acc = ps.tile([128, 1024], f32, tag="acc")
                    for n in range(2):
                        for k2 in range(2):
                            nc.tensor.matmul(
                                acc[:, n * 512 : (n + 1) * 512],
                                lt[:, 2 * k2 : 2 * k2 + 2, m * 128 : (m + 1) * 128],
                                rt[:, 2 * k2 : 2 * k2 + 2, n * 512 : (n + 1) * 512],
                                start=(k2 == 0), stop=(k2 == 1),
                                perf_mode=DR,
                            )
                    if ROUTE[m] == "a":
                        cp = cpp.tile([128, 1024], f16, tag="cpv")
                        nc.scalar.copy(cp[:], acc[:])
                        rm = run_min[na % 2]
                        na += 1
                        nc.vector.tensor_tensor(rm[:], rm[:], cp[:], MIN)
                    else:
                        nc.vector.tensor_reduce(
                            slots[:, nslot : nslot + 1], acc[:],
                            mybir.AxisListType.X, MIN,
                        )
                        nslot += 1

                # two bf16 diagonal graph blocks interleaved per sweep block
                # (no serial tail); copies go to VectorE which has headroom
                if 3 <= b <= 10:
                    for g in (2 * (b - 3), 2 * (b - 3) + 1):
                        dacc = psd.tile([128, G], f32, tag="dacc")
                        for k in range(KC):
                            nc.tensor.matmul(
                                dacc[:],
                                dgt[:, k, g * G : (g + 1) * G],
                                dgt[:, k, g * G : (g + 1) * G],
                                start=(k == 0), stop=(k == KC - 1),
                            )
                        dcp = small.tile([128, G], f32, tag="dcp")
                        nc.vector.tensor_copy(dcp[:], dacc[:])
                        nc.sync.dma_start(out=diag_out[g], in_=dcp[:])

            mm = small.tile([128, 3], f32, tag="mm")
            nc.vector.tensor_reduce(
                mm[:, 0:1], run_min[0][:], mybir.AxisListType.X, MIN
            )
            nc.vector.tensor_reduce(
                mm[:, 1:2], run_min[1][:], mybir.AxisListType.X, MIN
            )
            nc.vector.tensor_reduce(
                mm[:, 2:3], slots[:], mybir.AxisListType.X, MIN
            )
            nc.sync.dma_start(out=mins[:], in_=mm[:])

    nc.finalize()
    return nc


def _core_items(c: int):
    rows = [c, NHS - 1 - c]
    items = [(i, j) for i in rows for j in range(i, NHS)]
    assert len(items) == NBLK
    return items


def kernel(embeddings, row_idx, col_idx):
    global LAST_RESULTS
    from concourse.bass_utils import run_bass_kernel_spmd

    emb = np.asarray(embeddings, dtype=np.float32)
    row_idx = np.asarray(row_idx)
    col_idx = np.asarray(col_idx)

    if "nc" not in _CACHED:
        _CACHED["nc"] = _build_program()
    nc = _CACHED["nc"]

    eT = np.ascontiguousarray(emb.T)                       # [512, 16384] fp32
    e8 = eT.astype(ml_dtypes.float8_e4m3)                  # e4m3, RTNE
    # [slab, partition, d1, col]: per-partition-contiguous 4KB lines
    p8 = np.ascontiguousarray(
        e8.reshape(KC, 128, NHS, HS).transpose(2, 1, 0, 3)
    )
    e16 = eT.astype(ml_dtypes.bfloat16).reshape(KC, 128, NHS, HS)

    in_maps = []
    for c in range(NCORES):
        items = _core_items(c)
        lhs = p8[[i for i, _ in items]]
        rhs = p8[[j for _, j in items]]
        dg = np.ascontiguousarray(
            e16[:, :, [c, NHS - 1 - c], :].transpose(1, 0, 2, 3).reshape(
                128, KC, GPC * G
            )
        )
        in_maps.append({"lhs": lhs, "rhs": rhs, "dg": dg})

    res = run_bass_kernel_spmd(nc, in_maps, list(range(NCORES)))
    LAST_RESULTS = res

    m = min(r["mins"].min() for r in res.results)

    blocks = np.empty((N_GRAPHS, G, G), np.float32)
    gph = HS // G  # graphs per half-slab = 8
    for c in range(NCORES):
        rows = [c, NHS - 1 - c]
        gids = [i * gph + k for i in rows for k in range(gph)]
        for idx, g in enumerate(gids):
            blocks[g] = res.results[c]["diag_out"][idx]

    M = np.einsum("gii->gi", blocks).max()                 # global max (Cauchy-Schwarz)

    norm = (blocks - m) / (M - m + EPS)
    r = row_idx.astype(np.int64)
    cc = col_idx.astype(np.int64)
    out = norm[r >> 7, r & 127, cc & 127].astype(np.float32)
    return out


# revision 13
# speedup vs baseline: 1.3209x; 1.0229x over previous
"""LinkPredictor similarity kernel for 8 Trainium2 NeuronCores.

reference:
    sims = E @ E.T               # [16384, 16384], E = [16384, 512] fp32
    m, M = sims.min(), sims.max()
    sims = (sims - m) / (M - m + 1e-7)
    out  = sims[row_idx, col_idx]     # block-diag strict-upper-tri gather

Only the 128 diagonal [128,128] graph blocks are ever gathered, but the
global min needs every entry of sims. Two mathematical shortcuts:
  * sims is symmetric -> min over the block upper triangle suffices.
  * By Cauchy-Schwarz, s_ij <= |e_i||e_j| <= max_k |e_k|^2 = max diag,
    so the global max is exactly the max diagonal entry -- free from the
    bf16 diagonal blocks; the expensive sweep only tracks the min.

Distribution: 16 half-slabs of 1024 rows; core c owns half-slabs
{c, 15-c} and the 17 upper-triangle [1024,1024] blocks whose row
half-slab is one of those (every core gets exactly 17 blocks).

The sweep runs fp8e4 (e4m3) matmuls in DoubleRow perf mode (K=256 per
instruction): per [1024,1024] block, 8 two-bank PSUM pairs [128,1024]
are each filled by 4 matmuls. The running elementwise min is split to
keep both elementwise engines below the TensorE pace: per 8 pairs, 6 go
ScalarE-copy(fp16) -> VectorE running tensor_tensor min (two alternating
accumulators to break the RAW chain), 2 go VectorE tensor_reduce-min
directly on fp32 PSUM into independent slot columns. The 16 bf16
diagonal graph blocks run at the end (overlapping the VectorE drain)
and DMA straight out of PSUM. Host-simulated numerics: total pipeline
rel err ~1.6e-3 vs the 2e-2 gate.

Host combines mins, takes max from the block diagonals, normalizes and
gathers with the real row/col indices.
"""

import numpy as np
import ml_dtypes

N_GRAPHS = 128
G = 128
D = 512
N = N_GRAPHS * G          # 16384
EPS = 1e-7
NCORES = 8
HS = 1024                 # half-slab rows
NHS = N // HS             # 16 half-slabs
NBLK = 17                 # triangle blocks per core
KC = D // 128             # 4 contraction chunks of 128
MT = HS // 128            # 8 m-tiles per block
GPC = 16                  # graphs per core

_CACHED = {}
LAST_RESULTS = None       # BassKernelResults of the most recent run

# per-8-pair routing: 'a' = scalar->fp16->vector TT min, 'c' = vector
# tensor_reduce min direct on fp32 PSUM into a slot column
ROUTE = "aacaacaa"
NSLOTS = 64               # 2 'c' pairs x 17 blocks = 34 used


def _build_program():
    import concourse.bacc as bacc
    import concourse.mybir as mybir
    from concourse.tile import TileContext

    f32 = mybir.dt.float32
    f16 = mybir.dt.float16
    bf16 = mybir.dt.bfloat16
    f8 = mybir.dt.float8e4
    DR = mybir.MatmulPerfMode.DoubleRow
    MIN = mybir.AluOpType.min

    nc = bacc.Bacc(target_bir_lowering=False)
    # per-partition-contiguous packing: [block, partition, d1, col]
    lr = nc.declare_dram_parameter("lr", [NBLK, 128, KC, 2 * HS], f8, isOutput=False)
    dg_in = nc.declare_dram_parameter("dg", [128, KC, GPC * G], bf16, isOutput=False)
    diag_out = nc.declare_dram_parameter("diag_out", [GPC, G, G], f32, isOutput=True)
    mins = nc.declare_dram_parameter("mins", [128, 3], f32, isOutput=True)

    with TileContext(nc) as tc:
        with (
            tc.tile_pool(name="stream", bufs=1) as stream,
            tc.tile_pool(name="small", bufs=4) as small,
            tc.tile_pool(name="cpp", bufs=3) as cpp,
            tc.tile_pool(name="acc", bufs=1) as accp,
            tc.tile_pool(name="ps", bufs=3, space="PSUM") as ps,
            tc.tile_pool(name="psd", bufs=2, space="PSUM") as psd,
        ):
            run_min = [
                accp.tile([128, 1024], f16, tag=f"run_min{i}", name=f"run_min{i}")
                for i in range(2)
            ]
            slots = accp.tile([128, NSLOTS], f32, tag="slots")
            nc.vector.memset(run_min[0][:], 60000.0)
            nc.vector.memset(run_min[1][:], 60000.0)
            nc.vector.memset(slots[:], 3.0e38)

            dgt = accp.tile([128, KC, GPC * G], bf16, tag="dgt")

            # fully-resident streams: one buffer per block, every input DMA
            # issued up front on the SP ring -- no ring-reuse waits, nothing
            # ever blocks behind compute in an engine queue.
            lrs = []
            for b in range(NBLK):
                lrt = stream.tile(
                    [128, KC, 2 * HS], f8, tag=f"lr{b}", name=f"lr{b}"
                )
                lrs.append(lrt)
                if b == 0:
                    # fast start: land the m=0 operands first, the moving
                    # half via the ACT ring (its queue is empty this early)
                    nc.scalar.dma_start(
                        out=lrt[:, :, HS : 2 * HS], in_=lr[b][:, :, HS : 2 * HS]
                    )
                    nc.sync.dma_start(out=lrt[:, :, 0:256], in_=lr[b][:, :, 0:256])
                    nc.sync.dma_start(
                        out=lrt[:, :, 256:HS], in_=lr[b][:, :, 256:HS]
                    )
                else:
                    nc.sync.dma_start(out=lrt[:], in_=lr[b])
                if b == 2:
                    # diag operands: consumed only at the end
                    nc.sync.dma_start(out=dgt[:], in_=dg_in[:])

            na = 0
            nslot = 0
            for b in range(NBLK):
                lt = lrs[b]
                rt = lrs[b]
                for m in range(MT):
                    acc = ps.tile([128, 1024], f32, tag="acc")
                    for n in range(2):
                        for k2 in range(2):
                            nc.tensor.matmul(
                                acc[:, n * 512 : (n + 1) * 512],
                                lt[:, 2 * k2 : 2 * k2 + 2, m * 128 : (m + 1) * 128],
                                rt[
                                    :, 2 * k2 : 2 * k2 + 2,
                                    HS + n * 512 : HS + (n + 1) * 512,
                                ],
                                start=(k2 == 0), stop=(k2 == 1),
                                perf_mode=DR,
                            )
                    if ROUTE[m] == "a":
                        cp = cpp.tile([128, 1024], f16, tag="cpv")
                        nc.scalar.copy(cp[:], acc[:])
                        rm = run_min[na % 2]
                        na += 1
                        nc.vector.tensor_tensor(rm[:], rm[:], cp[:], MIN)
                    else:
                        nc.vector.tensor_reduce(
                            slots[:, nslot : nslot + 1], acc[:],
                            mybir.AxisListType.X, MIN,
                        )
                        nslot += 1

                # two bf16 diagonal graph blocks interleaved per sweep block
                # (no serial tail); copies go to VectorE which has headroom
                if 3 <= b <= 10:
                    for g in (2 * (b - 3), 2 * (b - 3) + 1):
                        dacc = psd.tile([128, G], f32, tag="dacc")
                        for k in range(KC):
                            nc.tensor.matmul(
                                dacc[:],
                                dgt[:, k, g * G : (g + 1) * G],
                                dgt[:, k, g * G : (g + 1) * G],
                                start=(k == 0), stop=(k == KC - 1),
                            )
                        dcp = small.tile([128, G], f32, tag="dcp")
                        nc.vector.tensor_copy(dcp[:], dacc[:])
                        nc.sync.dma_start(out=diag_out[g], in_=dcp[:])

            mm = small.tile([128, 3], f32, tag="mm")
            nc.vector.tensor_reduce(
                mm[:, 0:1], run_min[0][:], mybir.AxisListType.X, MIN
            )
            nc.vector.tensor_reduce(
                mm[:, 1:2], run_min[1][:], mybir.AxisListType.X, MIN
            )
            nc.vector.tensor_reduce(
                mm[:, 2:3], slots[:], mybir.AxisListType.X, MIN
            )
            nc.sync.dma_start(out=mins[:], in_=mm[:])

    nc.finalize()
    return nc


def _core_items(c: int):
    rows = [c, NHS - 1 - c]
    items = [(i, j) for i in rows for j in range(i, NHS)]
    assert len(items) == NBLK
    return items


def kernel(embeddings, row_idx, col_idx):
    global LAST_RESULTS
    from concourse.bass_utils import run_bass_kernel_spmd

    emb = np.asarray(embeddings, dtype=np.float32)
    row_idx = np.asarray(row_idx)
    col_idx = np.asarray(col_idx)

    if "nc" not in _CACHED:
        _CACHED["nc"] = _build_program()
    nc = _CACHED["nc"]

    eT = np.ascontiguousarray(emb.T)                       # [512, 16384] fp32
    e8 = eT.astype(ml_dtypes.float8_e4m3)                  # e4m3, RTNE
    # [slab, partition, d1, col]: per-partition-contiguous 4KB lines
    p8 = np.ascontiguousarray(
        e8.reshape(KC, 128, NHS, HS).transpose(2, 1, 0, 3)
    )
    e16 = eT.astype(ml_dtypes.bfloat16).reshape(KC, 128, NHS, HS)

    in_maps = []
    for c in range(NCORES):
        items = _core_items(c)
        lr = np.concatenate(
            [p8[[i for i, _ in items]], p8[[j for _, j in items]]], axis=3
        )
        dg = np.ascontiguousarray(
            e16[:, :, [c, NHS - 1 - c], :].transpose(1, 0, 2, 3).reshape(
                128, KC, GPC * G
            )
        )
        in_maps.append({"lr": lr, "dg": dg})

    res = run_bass_kernel_spmd(nc, in_maps, list(range(NCORES)))
    LAST_RESULTS = res

    m = min(r["mins"].min() for r in res.results)

    blocks = np.empty((N_GRAPHS, G, G), np.float32)
    gph = HS // G  # graphs per half-slab = 8
    for c in range(NCORES):
        rows = [c, NHS - 1 - c]
        gids = [i * gph + k for i in rows for k in range(gph)]
        for idx, g in enumerate(gids):
            blocks[g] = res.results[c]["diag_out"][idx]

    M = np.einsum("gii->gi", blocks).max()                 # global max (Cauchy-Schwarz)

    norm = (blocks - m) / (M - m + EPS)
    r = row_idx.astype(np.int64)
    cc = col_idx.astype(np.int64)
    out = norm[r >> 7, r & 127, cc & 127].astype(np.float32)
    return out
